# revision 23
# baseline (speedup 1.0000x reference)
"""Trainium2 Bass kernel for causal multi-head attention (dense transformer block).

Reference computation (per problem statement):
    qkv = x @ Wqkv.T ; split q,k,v ; RoPE(q), RoPE(k)
    scores = q @ k.T / sqrt(dh), causal mask, softmax
    o = probs @ v ; out = o @ Wo.T

Sharding: batch*heads across 8 cores (core c: batch c//4, heads 4*(c%4)..+4).
Each core computes its partial contribution to out (its heads through the
output projection); the host sums the 4 partials per batch at unshard time.

Device-side strategy (per core), designed so the ScalarE exp stream runs
saturated while the PE interleaves everything else around it:
  - q/k projected head-contiguous ([h: 32 even dims | 32 odd dims] on
    partitions), RoPE via cast + 32-row block-swap DMA + full-lane DVE ops
  - scores^T per 128-key j-tile as single-shot K=64 matmuls into 2-head
    f32 PSUM tiles; sc pool bufs=2 rotates so exp(jt) frees the slot
    exactly one ACT-slot before scores(jt+1) needs it -> ACT saturates
  - exp on ScalarE (no max subtraction; scores O(1) by construction), causal
    narrowing on diagonal tiles, triangular mask via DVE mul
  - PV via M=32 col-packed matmuls (head h dims 0:32 -> pvA[32h:32h+32],
    dims 32:64 -> pvB); softmax denominators via M=32 ones-matmuls land
    REPLICATED so normalization needs no partition broadcast (the baseline's
    broadcast DMA expanded to a ~12us packet train per block)
  - projection / v / output-projection units interleaved as PE filler
    inside the attention loops (1-2 per j-tile, paced to drain by block end)
"""

import numpy as np
import ml_dtypes

B, S, D = 2, 2048, 1024
H, DH = 16, 64
HALF = DH // 2            # 32
NCORES = 8
GPB = 4                   # cores (head-groups) per batch
HPC = H // GPB            # 4 heads per core
THETA = 10000.0
IB = 512                  # query block (free dim of scores^T)
NIB = S // IB             # 4
JT = 128                  # key tile (partition dim of scores^T)
NJT = S // JT             # 16
KT = 128                  # contraction tile
NKT = D // KT             # 8

BF16 = ml_dtypes.bfloat16

_NC_CACHE = None


def _build_nc():
    """Build + compile the (SPMD-identical) single-core Bass graph once."""
    global _NC_CACHE
    if _NC_CACHE is not None:
        return _NC_CACHE

    import concourse.bass as bass
    import concourse.mybir as mybir
    import concourse.tile as tile
    from concourse import bacc

    dt = mybir.dt
    bf = dt.bfloat16

    nc = bacc.Bacc("TRN2", target_bir_lowering=False, debug=False,
                   enable_asserts=False)

    xT = nc.dram_tensor("xT", [D, S], bf, kind="ExternalInput").ap()
    wqk = nc.dram_tensor("wqk", [D, 512], bf, kind="ExternalInput").ap()
    wv = nc.dram_tensor("wv", [D, HPC * DH], bf, kind="ExternalInput").ap()
    wo = nc.dram_tensor("wo", [HPC * DH, D], bf, kind="ExternalInput").ap()
    cosT = nc.dram_tensor("cosT", [128, S], bf, kind="ExternalInput").ap()
    sinT = nc.dram_tensor("sinT", [128, S], bf, kind="ExternalInput").ap()
    maskm = nc.dram_tensor("maskm", [JT, 2 * JT], bf, kind="ExternalInput").ap()
    out = nc.dram_tensor("out", [S, D], bf, kind="ExternalOutput").ap()

    with tile.TileContext(nc) as tc:
        _body(nc, tc, mybir, bass,
              xT, wqk, wv, wo, cosT, sinT, maskm, out)

    nc.compile()
    _NC_CACHE = nc
    return nc


def _body(nc, tc, mybir, bass, xT, wqk, wv, wo, cosT, sinT, maskm, out):
    dt = mybir.dt
    f32 = dt.float32
    bf = dt.bfloat16
    EXP = mybir.ActivationFunctionType.Exp

    from contextlib import ExitStack
    ctx = ExitStack()
    with ctx:
        consts = ctx.enter_context(tc.tile_pool(name="consts", bufs=1))
        persist = ctx.enter_context(tc.tile_pool(name="persist", bufs=1))
        ropet = ctx.enter_context(tc.tile_pool(name="ropet", bufs=4))
        prpool = ctx.enter_context(tc.tile_pool(name="prpool", bufs=6))
        recpool = ctx.enter_context(tc.tile_pool(name="recpool", bufs=2))
        stpool = ctx.enter_context(tc.tile_pool(name="stpool", bufs=4))
        # PSUM budget (8 banks): sc 2x2 + pv 2x1 + rs 1 + op 1
        ps = ctx.enter_context(tc.tile_pool(name="ps", bufs=2, space="PSUM"))
        psacc = ctx.enter_context(tc.tile_pool(name="psacc", bufs=2, space="PSUM"))
        psrs = ctx.enter_context(tc.tile_pool(name="psrs", bufs=1, space="PSUM"))
        psop = ctx.enter_context(tc.tile_pool(name="psop", bufs=1, space="PSUM"))

        # ---- constant/persistent SBUF loads -------------------------------
        # Each DMA doorbell occupies its issuing HWDGE queue for ~0.7us, so
        # loads alternate between the sync and scalar queues (scalar is idle
        # during the ramp; the exp stream starts much later). Later slabs
        # are paced into the early block loops via pend_dma.
        warm = consts.tile([128, 1], f32, tag="warm")
        nc.vector.memset(warm, 0.0)
        nc.scalar.activation(warm, warm, EXP)   # ACT exp table load, first

        _dmaq = [nc.sync, nc.scalar]
        _dmac = [0]

        def load(dst, src):
            _dmaq[_dmac[0] % 2].dma_start(out=dst, in_=src)
            _dmac[0] += 1

        xt = []
        wqkt = []
        for k in range(NKT):
            w = consts.tile([128, 512], bf, tag=f"wqk{k}", name=f"wqk{k}")
            wqkt.append(w)
            t = consts.tile([128, S], bf, tag=f"xt{k}", name=f"xt{k}")
            xt.append(t)
            load(w, wqk[k * 128:(k + 1) * 128, :])
            load(t[:, 0:IB], xT[k * 128:(k + 1) * 128, 0:IB])
        cos = consts.tile([128, S], bf, tag="cos")
        sin = consts.tile([128, S], bf, tag="sin")
        load(cos[:, 0:IB], cosT[:, 0:IB])
        load(sin[:, 0:IB], sinT[:, 0:IB])
        maskt = consts.tile([JT, 2, JT], bf, tag="maskt")
        load(maskt, maskm.rearrange("p (h f) -> p h f", h=2))
        wvt = []
        for k in range(NKT):
            t = consts.tile([128, HPC * DH], bf, tag=f"wv{k}", name=f"wv{k}")
            wvt.append(t)
            load(t, wv[k * 128:(k + 1) * 128, :])
        # block-1 slab eagerly too (needed by q1/k1 mid-block-0)
        for k in range(NKT):
            load(xt[k][:, IB:2 * IB], xT[k * 128:(k + 1) * 128, IB:2 * IB])
        load(cos[:, IB:2 * IB], cosT[:, IB:2 * IB])
        load(sin[:, IB:2 * IB], sinT[:, IB:2 * IB])
        wot = []
        for k in range(2):
            wot.append(consts.tile([128, D], bf, tag=f"wo{k}", name=f"wo{k}"))
        # remaining loads, paced into block b's loop and fully emitted there
        # (consumers -- q/k/v units for slab b+2 -- are emitted in block b+1,
        # AFTER these in program order, which is what defines dataflow)
        pend_dma_blocks = {0: [], 1: []}
        for k in range(2):
            pend_dma_blocks[0].append((0, wot[k], wo[k * 128:(k + 1) * 128, :]))
        for b in range(2, NIB):
            sl = slice(b * IB, (b + 1) * IB)
            dst_list = pend_dma_blocks[b - 2]
            for k in range(NKT):
                dst_list.append((k % 2, xt[k][:, sl],
                                 xT[k * 128:(k + 1) * 128, sl]))
            dst_list.append((0, cos[:, sl], cosT[:, sl]))
            dst_list.append((1, sin[:, sl], sinT[:, sl]))
        ones32 = consts.tile([128, 32], bf, tag="ones32")
        nc.vector.memset(ones32, 1.0)

        # persistent activations: head-pair-contiguous q/k, split-dim oT
        qR = [persist.tile([128, S], bf, tag=f"qR{t}", name=f"qR{t}") for t in range(2)]
        kR = [persist.tile([128, S], bf, tag=f"kR{t}", name=f"kR{t}") for t in range(2)]
        vbuf = persist.tile([128, NJT, HPC * DH], bf, tag="vbuf")
        oT = [persist.tile([128, S], bf, tag=f"oT{t}", name=f"oT{t}") for t in range(2)]

        # ---- interleavable PE units --------------------------------------
        def proj_unit(kind, b):
            # q or k projection (both head-pair tiles) for seq block b + RoPE.
            # wqk cols: [0:256]=q pairs, [256:512]=k pairs; within each 128:
            # [h: 32 even dims, 32 odd dims] x 2 heads (q pre-scaled)
            isl = slice(b * IB, (b + 1) * IB)
            base = 0 if kind == "q" else 256
            dst = qR if kind == "q" else kR
            pq = ps.tile([128, 2, IB], f32, tag="sc", name="pq")
            for t in range(2):
                fo = base + t * 128
                for k in range(NKT):
                    nc.tensor.matmul(pq[:, t, :],
                                     lhsT=wqkt[k][:, fo:fo + 128],
                                     rhs=xt[k][:, isl],
                                     start=(k == 0), stop=(k == NKT - 1))
            # rope: dst = pqb*cos + pqs*sinT (sinT rows carry -/+ signs);
            # pqs is pqb with 32-row blocks swapped (even<->odd halves of
            # each head) via the otherwise-idle gpsimd SWDGE queue
            rope_of(pq, dst, isl)

        def v_unit(b):
            # v projection for j-tiles 4b..4b+3 (natural [seq, dim] layout)
            vp = ps.tile([128, 4, 256], f32, tag="sc", name="vp")
            for q in range(4):
                jt = 4 * b + q
                for k in range(NKT):
                    nc.tensor.matmul(
                        vp[:, q, :],
                        lhsT=xt[k][:, jt * 128:(jt + 1) * 128],
                        rhs=wvt[k],
                        start=(k == 0), stop=(k == NKT - 1))
            nc.vector.tensor_copy(
                vbuf[:, 4 * b:4 * b + 4, :].rearrange("p a b -> p (a b)"),
                vp.rearrange("p a b -> p (a b)"))

        def op_unit(ic, mc, pool, tag, on_scalar):
            # output projection: out[ic*128:+128, mc*512:+512] partial
            icsl = slice(ic * 128, (ic + 1) * 128)
            msl = slice(mc * 512, (mc + 1) * 512)
            op = pool.tile([128, 512], f32, tag=tag, name="op")
            nc.tensor.matmul(op, lhsT=oT[0][:, icsl], rhs=wot[0][:, msl],
                             start=True, stop=False)
            nc.tensor.matmul(op, lhsT=oT[1][:, icsl], rhs=wot[1][:, msl],
                             start=False, stop=True)
            st = stpool.tile([128, 512], bf, tag="st", name="st")
            if on_scalar:
                nc.scalar.copy(out=st, in_=op)
            else:
                nc.vector.tensor_copy(st, op)
            nc.sync.dma_start(out=out[icsl, msl], in_=st)

        # ---- ramp: q/k for block 0, fused so the PE chases arriving
        # x/w tiles (MMs interleave q/k per contraction tile), then v -------
        def rope_of(pq, dst, isl):
            pqb = ropet.tile([128, 2, IB], bf, tag="rt", name="pqb")
            nc.vector.tensor_copy(pqb, pq)
            pqs = ropet.tile([128, 2, IB], bf, tag="rt", name="pqs")
            for blk in range(4):
                so = 32 * (blk ^ 1)
                nc.gpsimd.dma_start(out=pqs[32 * blk:32 * blk + 32, :, :],
                                    in_=pqb[so:so + 32, :, :])
            for t in range(2):
                t1 = ropet.tile([128, IB], bf, tag="rx", name="t1")
                t2 = ropet.tile([128, IB], bf, tag="rx", name="t2")
                nc.vector.tensor_mul(t1, pqb[:, t, :], cos[:, isl])
                nc.vector.tensor_mul(t2, pqs[:, t, :], sin[:, isl])
                nc.vector.tensor_add(dst[t][:, isl], t1, t2)

        isl0 = slice(0, IB)
        pq_q = ps.tile([128, 2, IB], f32, tag="sc", name="pq_q")
        pq_k = ps.tile([128, 2, IB], f32, tag="sc", name="pq_k")
        for k in range(NKT):
            for (pq, base) in ((pq_q, 0), (pq_k, 256)):
                for t in range(2):
                    nc.tensor.matmul(pq[:, t, :],
                                     lhsT=wqkt[k][:, base + t * 128:
                                                  base + t * 128 + 128],
                                     rhs=xt[k][:, isl0],
                                     start=(k == 0), stop=(k == NKT - 1))
        rope_of(pq_q, qR, isl0)
        rope_of(pq_k, kR, isl0)
        v_unit(0)

        # ---- attention blocks --------------------------------------------
        for b in range(NIB):
            isl = slice(b * IB, (b + 1) * IB)
            njt = (b + 1) * (IB // JT)
            # PE-filler unit queues for this block's loop:
            #  - sc-tag units (proj/v for block b+1), at most one per jt
            #  - op units (output projection for block b-1), one per jt
            pend_sc = []
            if b + 1 < NIB:
                pend_sc = [("q", b + 1), ("k", b + 1), ("v", b + 1)]
            pend_op = [(b - 1, j) for j in range(8)] if b > 0 else []
            pend_dma = pend_dma_blocks.get(b, [])

            pvA = psacc.tile([128, IB], f32, tag="acc", name="pvA")
            pvB = psacc.tile([128, IB], f32, tag="acc", name="pvB")
            rs = psrs.tile([128, IB], f32, tag="rs")

            def pv_rs(prs, d0, st0, last):
                # PV: M=32 col-packed (head h dims 0:32 -> pvA[32h:+32],
                # dims 32:64 -> pvB[32h:+32]); jt is captured via vj/prs
                for (dst, dh0) in ((pvA, 0), (pvB, 32)):
                    for h in range(4):
                        nc.tensor.matmul(
                            dst[32 * h:32 * h + 32, d0:IB],
                            lhsT=prs[4][:, 64 * h + dh0:64 * h + dh0 + 32],
                            rhs=prs[h // 2][:, h % 2, d0:IB],
                            start=st0, stop=last,
                            skip_group_check=True,
                            tile_position=(0, 32 * h))
                # row sums, replicated 32-wide per head (no broadcast later)
                for h in range(4):
                    nc.tensor.matmul(
                        rs[32 * h:32 * h + 32, d0:IB],
                        lhsT=ones32[:, 0:32],
                        rhs=prs[h // 2][:, h % 2, d0:IB],
                        start=st0, stop=last,
                        skip_group_check=True,
                        tile_position=(0, 32 * h))

            prev = None
            for jt in range(njt):
                jsl = slice(jt * JT, (jt + 1) * JT)
                delta = jt * JT - b * IB
                d0 = max(0, delta)
                qsl = slice(b * IB + d0, (b + 1) * IB)
                # scores: single-shot K=64 matmuls, 2 heads per sc tile
                scs = []
                for t in range(2):
                    sc = ps.tile([128, 2, IB], f32, tag="sc", name="sc")
                    for u in range(2):
                        nc.tensor.matmul(
                            sc[:, u, d0:IB],
                            lhsT=kR[t][64 * u:64 * u + 64, jsl],
                            rhs=qR[t][64 * u:64 * u + 64, qsl],
                            start=True, stop=True,
                            tile_position=(64 * u, 0))
                    scs.append(sc)
                # exp (+ diagonal triangular mask)
                prs = []
                for t in range(2):
                    pr = prpool.tile([128, 2, IB], bf, tag="pr", name="pr")
                    nc.scalar.activation(pr[:, :, d0:IB], scs[t][:, :, d0:IB],
                                         EXP)
                    if delta >= 0:
                        nc.vector.tensor_mul(pr[:, :, d0:d0 + JT],
                                             pr[:, :, d0:d0 + JT],
                                             maskt)
                    prs.append(pr)
                prs.append(None)
                prs.append(None)
                prs.append(vbuf[:, jt, :])
                # PE filler units while exp streams (these sit between
                # scores(jt) and pv(jt-1) in the PE queue, so the PE never
                # head-blocks on this jt's exp)
                if pend_sc:
                    kind, bb = pend_sc.pop(0)
                    proj_unit(kind, bb) if kind != "v" else v_unit(bb)
                if pend_op:
                    bb, j = pend_op.pop(0)
                    op_unit(4 * bb + j // 2, j % 2, psop, "op",
                            on_scalar=False)
                for _ in range(3):
                    if pend_dma:
                        qi, dq, sq = pend_dma.pop(0)
                        _dmaq[qi].dma_start(out=dq, in_=sq)
                # software pipeline: emit PREVIOUS jt's PV/rs behind this
                # jt's scores so the PE FIFO never stalls on exp(jt)
                if prev is not None:
                    pv_rs(*prev)
                prev = (prs, d0, jt == 0, jt == njt - 1)
            pv_rs(*prev)
            # drain leftovers (loads for slab b+2, projections for b+1)
            while pend_dma:
                qi, dq, sq = pend_dma.pop(0)
                _dmaq[qi].dma_start(out=dq, in_=sq)
            while pend_sc:
                kind, bb = pend_sc.pop(0)
                proj_unit(kind, bb) if kind != "v" else v_unit(bb)
            while pend_op:
                bb, j = pend_op.pop(0)
                op_unit(4 * bb + j // 2, j % 2, psop, "op", on_scalar=False)
            # normalization: oT = pv * (1/rs), partition-aligned
            rec = recpool.tile([128, IB], f32, tag="rec")
            nc.vector.reciprocal_approx_fast(out=rec, in_=rs)
            nc.vector.tensor_mul(oT[0][:, isl], pvA, rec)
            nc.vector.tensor_mul(oT[1][:, isl], pvB, rec)

        # drain the last block's output projection, alternating PSUM slots
        for j in range(8):
            if j % 2 == 0:
                op_unit(12 + j // 2, j % 2, psop, "op", on_scalar=True)
            else:
                op_unit(12 + j // 2, j % 2, psrs, "rs", on_scalar=False)


# ---------------------------------------------------------------------------
# Host-side sharding / unsharding
# ---------------------------------------------------------------------------

def _core_inputs(x, Wqkv, Wo, core):
    """Build the bf16 input map for one core (numpy, cheap)."""
    b = core // GPB
    heads = [HPC * (core % GPB) + j for j in range(HPC)]

    Wq = Wqkv[0 * D:1 * D]
    Wk = Wqkv[1 * D:2 * D]
    Wv = Wqkv[2 * D:3 * D]

    # head-contiguous, even dims then odd dims within each head
    rows_eo = [h * DH + 2 * t + p for h in heads
               for p in range(2) for t in range(HALF)]
    rows_v = [h * DH + d for h in heads for d in range(DH)]

    scale = 1.0 / np.sqrt(DH)
    wqk_host = np.concatenate([Wq[rows_eo] * scale, Wk[rows_eo]], axis=0)

    inv = THETA ** (-np.arange(HALF, dtype=np.float64) / HALF)
    ang = np.arange(S, dtype=np.float64)[None, :] * inv[:, None]   # [32, S]
    cos = np.tile(np.cos(ang), (4, 1))                             # [128, S]
    sn = np.sin(ang)
    sin_signed = np.concatenate([-sn, sn, -sn, sn], axis=0)        # [128, S]

    tri = (np.arange(JT)[None, :] >= np.arange(JT)[:, None]).astype(np.float32)
    maskm = np.tile(tri, (1, 2))                                   # [128, 256]

    # Wo rows: [h dims 0:32 for all h] then [h dims 32:64 for all h]
    woT = Wo[:, rows_v].T                                          # [256, 1024]
    rows_A = [h * DH + d for h in range(HPC) for d in range(32)]
    rows_B = [h * DH + 32 + d for h in range(HPC) for d in range(32)]
    wo_host = np.concatenate([woT[rows_A], woT[rows_B]], axis=0)

    c = lambda a: np.ascontiguousarray(a).astype(BF16)
    return {
        "xT": c(x[b].T),
        "wqk": c(wqk_host.T),
        "wv": c(Wv[rows_v].T),
        "wo": c(wo_host),
        "cosT": c(cos),
        "sinT": c(sin_signed),
        "maskm": c(maskm),
    }


def _run(x, Wqkv, Wo, trace=False):
    nc = _build_nc()
    from concourse.bass_utils import run_bass_kernel_spmd
    in_maps = [_core_inputs(x, Wqkv, Wo, c) for c in range(NCORES)]
    res = run_bass_kernel_spmd(nc, in_maps, core_ids=list(range(NCORES)),
                               trace=trace)
    parts = [res.results[i]["out"].astype(np.float32) for i in range(NCORES)]
    full = np.stack([sum(parts[0:GPB]), sum(parts[GPB:2 * GPB])], axis=0)
    return full, res


def kernel(x, Wqkv, Wo):
    x = np.asarray(x, dtype=np.float32)
    Wqkv = np.asarray(Wqkv, dtype=np.float32)
    Wo = np.asarray(Wo, dtype=np.float32)
    full, _ = _run(x, Wqkv, Wo, trace=False)
    return full


# revision 24
# speedup vs baseline: 1.1994x; 1.1994x over previous
"""Trainium2 Bass kernel for causal multi-head attention (dense transformer block).

Reference computation (per problem statement):
    qkv = x @ Wqkv.T ; split q,k,v ; RoPE(q), RoPE(k)
    scores = q @ k.T / sqrt(dh), causal mask, softmax
    o = probs @ v ; out = o @ Wo.T

Sharding: batch*heads across 8 cores (core c: batch c//4, heads 4*(c%4)..+4).
Each core computes its partial contribution to out (its heads through the
output projection); the host sums the 4 partials per batch at unshard time.

Device-side strategy (per core), designed so the ScalarE exp stream runs
saturated while the PE interleaves everything else around it:
  - q/k projected head-contiguous ([h: 32 even dims | 32 odd dims] on
    partitions), RoPE via cast + 32-row block-swap DMA + full-lane DVE ops
  - scores^T per 128-key j-tile as single-shot K=64 matmuls into 2-head
    f32 PSUM tiles; sc pool bufs=2 rotates so exp(jt) frees the slot
    exactly one ACT-slot before scores(jt+1) needs it -> ACT saturates
  - exp on ScalarE (no max subtraction; scores O(1) by construction), causal
    narrowing on diagonal tiles, triangular mask via DVE mul
  - PV via M=32 col-packed matmuls (head h dims 0:32 -> pvA[32h:32h+32],
    dims 32:64 -> pvB); softmax denominators via M=32 ones-matmuls land
    REPLICATED so normalization needs no partition broadcast (the baseline's
    broadcast DMA expanded to a ~12us packet train per block)
  - projection / v / output-projection units interleaved as PE filler
    inside the attention loops (1-2 per j-tile, paced to drain by block end)
"""

import numpy as np
import ml_dtypes

B, S, D = 2, 2048, 1024
H, DH = 16, 64
HALF = DH // 2            # 32
NCORES = 8
GPB = 4                   # cores (head-groups) per batch
HPC = H // GPB            # 4 heads per core
THETA = 10000.0
IB = 512                  # query block (free dim of scores^T)
NIB = S // IB             # 4
JT = 128                  # key tile (partition dim of scores^T)
NJT = S // JT             # 16
KT = 128                  # contraction tile
NKT = D // KT             # 8

BF16 = ml_dtypes.bfloat16

_NC_CACHE = None


def _build_nc():
    """Build + compile the (SPMD-identical) single-core Bass graph once."""
    global _NC_CACHE
    if _NC_CACHE is not None:
        return _NC_CACHE

    import concourse.bass as bass
    import concourse.mybir as mybir
    import concourse.tile as tile
    from concourse import bacc

    dt = mybir.dt
    bf = dt.bfloat16

    nc = bacc.Bacc("TRN2", target_bir_lowering=False, debug=False,
                   enable_asserts=False)

    xT = nc.dram_tensor("xT", [D, S], bf, kind="ExternalInput").ap()
    wqk = nc.dram_tensor("wqk", [D, 512], bf, kind="ExternalInput").ap()
    wv = nc.dram_tensor("wv", [D, HPC * DH], bf, kind="ExternalInput").ap()
    wo = nc.dram_tensor("wo", [HPC * DH, D], bf, kind="ExternalInput").ap()
    cosT = nc.dram_tensor("cosT", [128, S], bf, kind="ExternalInput").ap()
    sinT = nc.dram_tensor("sinT", [128, S], bf, kind="ExternalInput").ap()
    maskm = nc.dram_tensor("maskm", [JT, 2 * JT], bf, kind="ExternalInput").ap()
    out = nc.dram_tensor("out", [S, D], bf, kind="ExternalOutput").ap()

    with tile.TileContext(nc) as tc:
        _body(nc, tc, mybir, bass,
              xT, wqk, wv, wo, cosT, sinT, maskm, out)

    nc.compile()
    _NC_CACHE = nc
    return nc


def _body(nc, tc, mybir, bass, xT, wqk, wv, wo, cosT, sinT, maskm, out):
    dt = mybir.dt
    f32 = dt.float32
    bf = dt.bfloat16
    EXP = mybir.ActivationFunctionType.Exp

    from contextlib import ExitStack
    ctx = ExitStack()
    with ctx:
        consts = ctx.enter_context(tc.tile_pool(name="consts", bufs=1))
        persist = ctx.enter_context(tc.tile_pool(name="persist", bufs=1))
        ropet = ctx.enter_context(tc.tile_pool(name="ropet", bufs=4))
        prpool = ctx.enter_context(tc.tile_pool(name="prpool", bufs=6))
        recpool = ctx.enter_context(tc.tile_pool(name="recpool", bufs=2))
        stpool = ctx.enter_context(tc.tile_pool(name="stpool", bufs=4))
        # PSUM budget (8 banks): sc 2x2 + pv 2x1 + rs 1 + op 1
        ps = ctx.enter_context(tc.tile_pool(name="ps", bufs=2, space="PSUM"))
        psacc = ctx.enter_context(tc.tile_pool(name="psacc", bufs=2, space="PSUM"))
        psrs = ctx.enter_context(tc.tile_pool(name="psrs", bufs=1, space="PSUM"))
        psop = ctx.enter_context(tc.tile_pool(name="psop", bufs=1, space="PSUM"))

        # ---- constant/persistent SBUF loads -------------------------------
        # Each DMA doorbell occupies its issuing HWDGE queue for ~0.7us, so
        # loads alternate between the sync and scalar queues (scalar is idle
        # during the ramp; the exp stream starts much later). Later slabs
        # are paced into the early block loops via pend_dma.
        warm = consts.tile([128, 1], f32, tag="warm")
        nc.vector.memset(warm, 0.0)
        nc.scalar.activation(warm, warm, EXP)   # ACT exp table load, first

        _dmaq = [nc.sync, nc.scalar]
        _dmac = [0]

        def load(dst, src):
            _dmaq[_dmac[0] % 2].dma_start(out=dst, in_=src)
            _dmac[0] += 1

        xt = []
        wqkt = []
        for k in range(NKT):
            w = consts.tile([128, 512], bf, tag=f"wqk{k}", name=f"wqk{k}")
            wqkt.append(w)
            t = consts.tile([128, S], bf, tag=f"xt{k}", name=f"xt{k}")
            xt.append(t)
            load(w, wqk[k * 128:(k + 1) * 128, :])
            load(t[:, 0:IB], xT[k * 128:(k + 1) * 128, 0:IB])
        cos = consts.tile([128, S], bf, tag="cos")
        sin = consts.tile([128, S], bf, tag="sin")
        load(cos[:, 0:IB], cosT[:, 0:IB])
        load(sin[:, 0:IB], sinT[:, 0:IB])
        maskt = consts.tile([JT, 2, JT], bf, tag="maskt")
        load(maskt, maskm.rearrange("p (h f) -> p h f", h=2))
        wvt = []
        for k in range(NKT):
            t = consts.tile([128, HPC * DH], bf, tag=f"wv{k}", name=f"wv{k}")
            wvt.append(t)
            load(t, wv[k * 128:(k + 1) * 128, :])
        # block-1 slab eagerly too (needed by q1/k1 mid-block-0)
        for k in range(NKT):
            load(xt[k][:, IB:2 * IB], xT[k * 128:(k + 1) * 128, IB:2 * IB])
        load(cos[:, IB:2 * IB], cosT[:, IB:2 * IB])
        load(sin[:, IB:2 * IB], sinT[:, IB:2 * IB])
        wot = []
        for k in range(2):
            wot.append(consts.tile([128, D], bf, tag=f"wo{k}", name=f"wo{k}"))
        # remaining loads, paced into block b's loop and fully emitted there
        # (consumers -- q/k/v units for slab b+2 -- are emitted in block b+1,
        # AFTER these in program order, which is what defines dataflow)
        pend_dma_blocks = {0: [], 1: []}
        for k in range(2):
            pend_dma_blocks[0].append((0, wot[k], wo[k * 128:(k + 1) * 128, :]))
        for b in range(2, NIB):
            sl = slice(b * IB, (b + 1) * IB)
            dst_list = pend_dma_blocks[b - 2]
            for k in range(NKT):
                dst_list.append((k % 2 if b == 2 else 0, xt[k][:, sl],
                                 xT[k * 128:(k + 1) * 128, sl]))
            dst_list.append((0, cos[:, sl], cosT[:, sl]))
            dst_list.append((1 if b == 2 else 0, sin[:, sl], sinT[:, sl]))
        ones32 = consts.tile([128, 32], bf, tag="ones32")
        nc.vector.memset(ones32, 1.0)

        # persistent activations: head-pair-contiguous q/k, split-dim oT
        qR = [persist.tile([128, S], bf, tag=f"qR{t}", name=f"qR{t}") for t in range(2)]
        kR = [persist.tile([128, S], bf, tag=f"kR{t}", name=f"kR{t}") for t in range(2)]
        vbuf = persist.tile([128, NJT, HPC * DH], bf, tag="vbuf")
        oT = [persist.tile([128, S], bf, tag=f"oT{t}", name=f"oT{t}") for t in range(2)]

        # ---- interleavable PE units --------------------------------------
        def proj_unit(kind, b):
            # q or k projection (both head-pair tiles) for seq block b + RoPE.
            # wqk cols: [0:256]=q pairs, [256:512]=k pairs; within each 128:
            # [h: 32 even dims, 32 odd dims] x 2 heads (q pre-scaled)
            isl = slice(b * IB, (b + 1) * IB)
            base = 0 if kind == "q" else 256
            dst = qR if kind == "q" else kR
            pq = ps.tile([128, 2, IB], f32, tag="sc", name="pq")
            for t in range(2):
                fo = base + t * 128
                for k in range(NKT):
                    nc.tensor.matmul(pq[:, t, :],
                                     lhsT=wqkt[k][:, fo:fo + 128],
                                     rhs=xt[k][:, isl],
                                     start=(k == 0), stop=(k == NKT - 1))
            # rope: dst = pqb*cos + pqs*sinT (sinT rows carry -/+ signs);
            # pqs is pqb with 32-row blocks swapped (even<->odd halves of
            # each head) via the otherwise-idle gpsimd SWDGE queue
            rope_of(pq, dst, isl)

        def v_unit(b):
            # v projection for j-tiles 4b..4b+3 (natural [seq, dim] layout)
            vp = ps.tile([128, 4, 256], f32, tag="sc", name="vp")
            for q in range(4):
                jt = 4 * b + q
                for k in range(NKT):
                    nc.tensor.matmul(
                        vp[:, q, :],
                        lhsT=xt[k][:, jt * 128:(jt + 1) * 128],
                        rhs=wvt[k],
                        start=(k == 0), stop=(k == NKT - 1))
            nc.vector.tensor_copy(
                vbuf[:, 4 * b:4 * b + 4, :].rearrange("p a b -> p (a b)"),
                vp.rearrange("p a b -> p (a b)"))

        def op_unit(ic, mc, pool, tag, on_scalar):
            # output projection: out[ic*128:+128, mc*512:+512] partial
            icsl = slice(ic * 128, (ic + 1) * 128)
            msl = slice(mc * 512, (mc + 1) * 512)
            op = pool.tile([128, 512], f32, tag=tag, name="op")
            nc.tensor.matmul(op, lhsT=oT[0][:, icsl], rhs=wot[0][:, msl],
                             start=True, stop=False)
            nc.tensor.matmul(op, lhsT=oT[1][:, icsl], rhs=wot[1][:, msl],
                             start=False, stop=True)
            st = stpool.tile([128, 512], bf, tag="st", name="st")
            if on_scalar:
                nc.scalar.copy(out=st, in_=op)
            else:
                nc.vector.tensor_copy(st, op)
            nc.sync.dma_start(out=out[icsl, msl], in_=st)

        # ---- ramp: q/k for block 0, fused so the PE chases arriving
        # x/w tiles (MMs interleave q/k per contraction tile), then v -------
        def rope_of(pq, dst, isl):
            pqb = ropet.tile([128, 2, IB], bf, tag="rt", name="pqb")
            nc.vector.tensor_copy(pqb, pq)
            pqs = ropet.tile([128, 2, IB], bf, tag="rt", name="pqs")
            for blk in range(4):
                so = 32 * (blk ^ 1)
                nc.gpsimd.dma_start(out=pqs[32 * blk:32 * blk + 32, :, :],
                                    in_=pqb[so:so + 32, :, :])
            for t in range(2):
                t1 = ropet.tile([128, IB], bf, tag="rx", name="t1")
                t2 = ropet.tile([128, IB], bf, tag="rx", name="t2")
                nc.vector.tensor_mul(t1, pqb[:, t, :], cos[:, isl])
                nc.vector.tensor_mul(t2, pqs[:, t, :], sin[:, isl])
                nc.vector.tensor_add(dst[t][:, isl], t1, t2)

        isl0 = slice(0, IB)
        pq_q = ps.tile([128, 2, IB], f32, tag="sc", name="pq_q")
        pq_k = ps.tile([128, 2, IB], f32, tag="sc", name="pq_k")
        for k in range(NKT):
            for (pq, base) in ((pq_q, 0), (pq_k, 256)):
                for t in range(2):
                    nc.tensor.matmul(pq[:, t, :],
                                     lhsT=wqkt[k][:, base + t * 128:
                                                  base + t * 128 + 128],
                                     rhs=xt[k][:, isl0],
                                     start=(k == 0), stop=(k == NKT - 1))
        rope_of(pq_q, qR, isl0)
        rope_of(pq_k, kR, isl0)
        v_unit(0)

        # ---- attention blocks --------------------------------------------
        for b in range(NIB):
            isl = slice(b * IB, (b + 1) * IB)
            njt = (b + 1) * (IB // JT)
            # PE-filler unit queues for this block's loop:
            #  - sc-tag units (proj/v for block b+1), at most one per jt
            #  - op units (output projection for block b-1), one per jt
            pend_sc = []
            if b + 1 < NIB:
                pend_sc = [("q", b + 1), ("k", b + 1), ("v", b + 1)]
            pend_op = [(b - 1, j) for j in range(8)] if b > 0 else []
            pend_dma = pend_dma_blocks.get(b, [])

            pvA = psacc.tile([128, IB], f32, tag="acc", name="pvA")
            pvB = psacc.tile([128, IB], f32, tag="acc", name="pvB")
            rs = psrs.tile([128, IB], f32, tag="rs")

            def pv_rs(prs, d0, st0, last):
                # PV: M=32 col-packed (head h dims 0:32 -> pvA[32h:+32],
                # dims 32:64 -> pvB[32h:+32]); jt is captured via vj/prs
                for (dst, dh0) in ((pvA, 0), (pvB, 32)):
                    for h in range(4):
                        nc.tensor.matmul(
                            dst[32 * h:32 * h + 32, d0:IB],
                            lhsT=prs[4][:, 64 * h + dh0:64 * h + dh0 + 32],
                            rhs=prs[h // 2][:, h % 2, d0:IB],
                            start=st0, stop=last,
                            skip_group_check=True,
                            tile_position=(0, 32 * h))
                # row sums, replicated 32-wide per head (no broadcast later)
                for h in range(4):
                    nc.tensor.matmul(
                        rs[32 * h:32 * h + 32, d0:IB],
                        lhsT=ones32[:, 0:32],
                        rhs=prs[h // 2][:, h % 2, d0:IB],
                        start=st0, stop=last,
                        skip_group_check=True,
                        tile_position=(0, 32 * h))

            prev = None
            for jt in range(njt):
                jsl = slice(jt * JT, (jt + 1) * JT)
                delta = jt * JT - b * IB
                d0 = max(0, delta)
                qsl = slice(b * IB + d0, (b + 1) * IB)
                # scores: single-shot K=64 matmuls, 2 heads per sc tile
                scs = []
                for t in range(2):
                    sc = ps.tile([128, 2, IB], f32, tag="sc", name="sc")
                    for u in range(2):
                        nc.tensor.matmul(
                            sc[:, u, d0:IB],
                            lhsT=kR[t][64 * u:64 * u + 64, jsl],
                            rhs=qR[t][64 * u:64 * u + 64, qsl],
                            start=True, stop=True,
                            tile_position=(64 * u, 0))
                    scs.append(sc)
                # exp (+ diagonal triangular mask)
                prs = []
                for t in range(2):
                    pr = prpool.tile([128, 2, IB], bf, tag="pr", name="pr")
                    nc.scalar.activation(pr[:, :, d0:IB], scs[t][:, :, d0:IB],
                                         EXP)
                    if delta >= 0:
                        nc.vector.tensor_mul(pr[:, :, d0:d0 + JT],
                                             pr[:, :, d0:d0 + JT],
                                             maskt)
                    prs.append(pr)
                prs.append(None)
                prs.append(None)
                prs.append(vbuf[:, jt, :])
                # PE filler units while exp streams (these sit between
                # scores(jt) and pv(jt-1) in the PE queue, so the PE never
                # head-blocks on this jt's exp)
                if pend_sc:
                    kind, bb = pend_sc.pop(0)
                    proj_unit(kind, bb) if kind != "v" else v_unit(bb)
                if pend_op:
                    bb, j = pend_op.pop(0)
                    op_unit(4 * bb + j // 2, j % 2, psop, "op",
                            on_scalar=False)
                for _ in range(3):
                    if pend_dma:
                        qi, dq, sq = pend_dma.pop(0)
                        _dmaq[qi].dma_start(out=dq, in_=sq)
                # software pipeline: emit PREVIOUS jt's PV/rs behind this
                # jt's scores so the PE FIFO never stalls on exp(jt)
                if prev is not None:
                    pv_rs(*prev)
                prev = (prs, d0, jt == 0, jt == njt - 1)
            pv_rs(*prev)
            # drain leftovers (loads for slab b+2, projections for b+1)
            while pend_dma:
                qi, dq, sq = pend_dma.pop(0)
                _dmaq[qi].dma_start(out=dq, in_=sq)
            while pend_sc:
                kind, bb = pend_sc.pop(0)
                proj_unit(kind, bb) if kind != "v" else v_unit(bb)
            while pend_op:
                bb, j = pend_op.pop(0)
                op_unit(4 * bb + j // 2, j % 2, psop, "op", on_scalar=False)
            # normalization: oT = pv * (1/rs), partition-aligned
            rec = recpool.tile([128, IB], f32, tag="rec")
            nc.vector.reciprocal_approx_fast(out=rec, in_=rs)
            nc.vector.tensor_mul(oT[0][:, isl], pvA, rec)
            nc.vector.tensor_mul(oT[1][:, isl], pvB, rec)

        # drain the last block's output projection, alternating PSUM slots
        for j in range(8):
            if j % 2 == 0:
                op_unit(12 + j // 2, j % 2, psop, "op", on_scalar=True)
            else:
                op_unit(12 + j // 2, j % 2, psrs, "rs", on_scalar=False)


# ---------------------------------------------------------------------------
# Host-side sharding / unsharding
# ---------------------------------------------------------------------------

def _core_inputs(x, Wqkv, Wo, core):
    """Build the bf16 input map for one core (numpy, cheap)."""
    b = core // GPB
    heads = [HPC * (core % GPB) + j for j in range(HPC)]

    Wq = Wqkv[0 * D:1 * D]
    Wk = Wqkv[1 * D:2 * D]
    Wv = Wqkv[2 * D:3 * D]

    # head-contiguous, even dims then odd dims within each head
    rows_eo = [h * DH + 2 * t + p for h in heads
               for p in range(2) for t in range(HALF)]
    rows_v = [h * DH + d for h in heads for d in range(DH)]

    scale = 1.0 / np.sqrt(DH)
    wqk_host = np.concatenate([Wq[rows_eo] * scale, Wk[rows_eo]], axis=0)

    inv = THETA ** (-np.arange(HALF, dtype=np.float64) / HALF)
    ang = np.arange(S, dtype=np.float64)[None, :] * inv[:, None]   # [32, S]
    cos = np.tile(np.cos(ang), (4, 1))                             # [128, S]
    sn = np.sin(ang)
    sin_signed = np.concatenate([-sn, sn, -sn, sn], axis=0)        # [128, S]

    tri = (np.arange(JT)[None, :] >= np.arange(JT)[:, None]).astype(np.float32)
    maskm = np.tile(tri, (1, 2))                                   # [128, 256]

    # Wo rows: [h dims 0:32 for all h] then [h dims 32:64 for all h]
    woT = Wo[:, rows_v].T                                          # [256, 1024]
    rows_A = [h * DH + d for h in range(HPC) for d in range(32)]
    rows_B = [h * DH + 32 + d for h in range(HPC) for d in range(32)]
    wo_host = np.concatenate([woT[rows_A], woT[rows_B]], axis=0)

    c = lambda a: np.ascontiguousarray(a).astype(BF16)
    return {
        "xT": c(x[b].T),
        "wqk": c(wqk_host.T),
        "wv": c(Wv[rows_v].T),
        "wo": c(wo_host),
        "cosT": c(cos),
        "sinT": c(sin_signed),
        "maskm": c(maskm),
    }


def _run(x, Wqkv, Wo, trace=False):
    nc = _build_nc()
    from concourse.bass_utils import run_bass_kernel_spmd
    in_maps = [_core_inputs(x, Wqkv, Wo, c) for c in range(NCORES)]
    res = run_bass_kernel_spmd(nc, in_maps, core_ids=list(range(NCORES)),
                               trace=trace)
    parts = [res.results[i]["out"].astype(np.float32) for i in range(NCORES)]
    full = np.stack([sum(parts[0:GPB]), sum(parts[GPB:2 * GPB])], axis=0)
    return full, res


def kernel(x, Wqkv, Wo):
    x = np.asarray(x, dtype=np.float32)
    Wqkv = np.asarray(Wqkv, dtype=np.float32)
    Wo = np.asarray(Wo, dtype=np.float32)
    full, _ = _run(x, Wqkv, Wo, trace=False)
    return full
